# revision 24
# baseline (speedup 1.0000x reference)
"""Binarized Conv1d + BatchNorm1d (training mode) on 8 TRN2 NeuronCores.

Reference computation:
    bx  = sign(x)          [B=16, Cin=128, L=8192]
    bw  = sign(weight)     [Cout=128, Cin=128, K=5]
    out = conv1d(bx, bw, stride=1, pad=2) + bias
    out = (out - mean(out, (B,L))) * rsqrt(var(out, (B,L)) + 1e-5)

Sharding: data-parallel over batch, 2 batches per core; weights replicated.

Key tricks vs the straightforward version:
  - half-sign encoding: h = (x>0) - 0.5 in {-0.5,+0.5} (one DVE
    tensor_scalar op; exact since x has no zeros; conv pads = 0).  Then
    conv(h, bw) = M/2 where M is the true conv, and training-mode BN
    absorbs the scale: out = (M/2 - mean_{M/2}) * rsqrt(var_{M/2} + EPS/4).
    Binarize runs entirely on DVE; ACT only does PSUM drains + the final
    sqrt, all functions from the single sqrt_and_others ACT table (no
    mid-kernel ACT_TABLE_LOAD).
  - weights are sign()ed/transposed to [ci,k,co] bf16 on the host.
  - stats all-reduce via a [2,128]-transposed AllGather: the gathered
    [16,128] reduces with one tiny matmul (no 1024-descriptor DMAs).
"""

import os
import sys

import numpy as np

try:
    import concourse  # noqa: F401
except ImportError:
    for _p in ("/opt/trn_rl_repo", "/root/.axon_site/_ro/trn_rl_repo"):
        if os.path.isdir(_p):
            sys.path.insert(0, _p)
            break

B = 16
B_LOC = 2
CI = 128
CO = 128
L = 8192
K = 5
PAD = 2
EPS = 1e-5
N_CORES = 8
FREE = 512          # PSUM tile free dim (one bank of f32)
NT = L // FREE      # 16 conv tiles per batch row

_CACHE = {}


def _build_nc():
    import concourse.bacc as bacc
    import concourse.tile as tile
    from concourse import mybir

    f32 = mybir.dt.float32
    bf16 = mybir.dt.bfloat16
    Sqrt = mybir.ActivationFunctionType.Sqrt
    Copy = mybir.ActivationFunctionType.Copy
    Ident = mybir.ActivationFunctionType.Identity
    ALU = mybir.AluOpType

    nc = bacc.Bacc("TRN2", target_bir_lowering=False, debug=False, num_devices=N_CORES)

    x = nc.declare_dram_parameter("x", [B_LOC, CI, L], f32, isOutput=False)
    wT = nc.declare_dram_parameter("wT", [CI, K, CO], bf16, isOutput=False)
    idm = nc.declare_dram_parameter("ident", [128, 128], f32, isOutput=False)
    msk = nc.declare_dram_parameter("mask16", [16, 2], f32, isOutput=False)
    out = nc.declare_dram_parameter("out", [B_LOC, CO, L], f32, isOutput=True)

    with tile.TileContext(nc) as tc:
        with (
            tc.tile_pool(name="singles", bufs=1) as singles,
            tc.tile_pool(name="xin", bufs=1) as xin,
            tc.tile_pool(name="bxp", bufs=2) as bxp_pool,
            tc.tile_pool(name="psum", bufs=8, space="PSUM") as psum,
            tc.tile_pool(name="dram", bufs=1, space="DRAM") as dram,
            tc.tile_pool(name="warm_dram", bufs=1, space="DRAM") as wdram,
        ):
            # ---- warm-up collective, fired first: the CC firmware takes
            # ~55us to boot after NEFF start and the first collective pays
            # ~11us of cold handling; the warmup absorbs both during the
            # conv phase so the real stats CC only pays ~5-6us ----
            # one warmup: eats the CC firmware cold handling + boot-skew
            # rendezvous during the conv phase (a second warmup measured
            # strictly worse: 129-191us samples vs 112)
            for wi in range(1):
                warm_in = wdram.tile([1, 8], f32, name=f"warm_in{wi}")
                warm_out = wdram.tile([N_CORES, 8], f32, name=f"warm_out{wi}")
                nc.gpsimd.dma_start(out=warm_in, in_=idm[wi : wi + 1, 0:8])
                nc.gpsimd.collective_compute(
                    "AllGather",
                    mybir.AluOpType.bypass,
                    replica_groups=[list(range(N_CORES))],
                    ins=[warm_in[:].opt()],
                    outs=[warm_out[:].opt()],
                )

            # ---- constants + weights + x streamed in (weights first so the
            # first matmul group can start as soon as chunk 0 lands) ----
            xts = [
                xin.tile([CI, L], f32, tag=f"xt{b}", name=f"xt{b}")
                for b in range(B_LOC)
            ]
            wTt = singles.tile([CI, K, CO], bf16)
            nc.sync.dma_start(out=wTt, in_=wT[:, :, :])
            # chunk boundaries chosen so each ready-tile group is <=4
            # (8 PSUM banks: group + draining predecessor stay in flight);
            # small leading chunks so the first matmuls start early
            CH_SCHED = [520, 512, 2048, 2048, 1024, 1024, 512, 504]
            CHUNKS = {0: CH_SCHED, 1: CH_SCHED}
            ident = singles.tile([128, 128], f32)
            for b in range(B_LOC):
                off = 0
                for ch in CHUNKS[b]:
                    nc.sync.dma_start(
                        out=xts[b][:, off : off + ch],
                        in_=x[b, :, off : off + ch],
                    )
                    off += ch
                if b == 0:
                    # ident is only needed for the stats transpose at conv
                    # end; keep it off the startup critical path
                    nc.sync.dma_start(out=ident, in_=idm[:, :])
            mask16 = singles.tile([16, 2], f32)
            nc.sync.dma_start(out=mask16, in_=msk[:, :])

            # ---- conv: binarize (half-sign encoding) + bf16 matmuls ----
            conv_sb = singles.tile([CO, B_LOC, L], f32)
            stats = singles.tile([CO, B_LOC * NT, 6], f32)

            for b in range(B_LOC):
                bxp = bxp_pool.tile([CI, L + 2 * PAD], bf16)
                nc.vector.memset(bxp[:, 0:PAD], 0.0)
                nc.vector.memset(bxp[:, L + PAD : L + 2 * PAD], 0.0)
                xt = xts[b]
                done_t = 0
                off = 0
                for ch in CHUNKS[b]:
                    # h = (x > 0) - 0.5 in {-0.5, +0.5}: one DVE op
                    nc.vector.tensor_scalar(
                        out=bxp[:, PAD + off : PAD + off + ch],
                        in0=xt[:, off : off + ch],
                        scalar1=0.0, scalar2=0.5,
                        op0=ALU.is_gt, op1=ALU.subtract,
                    )
                    off += ch
                    # conv tiles fully covered by binarized cols [0, off):
                    # tile t needs bxp up to index t*512+515; filled thru
                    # 2+off-1 (plus right pad once off==L)
                    lim = off + PAD - 1 + (PAD if off == L else 0)
                    group = []
                    while done_t < NT and done_t * FREE + 515 <= lim:
                        group.append(done_t)
                        done_t += 1
                    if not group:
                        continue
                    # k-outer over the group amortizes PE LoadStationary
                    pts = {}
                    for t in group:
                        pts[t] = psum.tile(
                            [CO, FREE], f32, tag="pt", name=f"pt{b}_{t}"
                        )
                    for k in range(K):
                        for t in group:
                            nc.tensor.matmul(
                                pts[t], lhsT=wTt[:, k, :],
                                rhs=bxp[:, t * FREE + k : t * FREE + k + FREE],
                                start=(k == 0), stop=(k == K - 1),
                            )
                    for t in group:
                        nc.vector.bn_stats(out=stats[:, b * NT + t, :], in_=pts[t])
                        nc.scalar.activation(
                            out=conv_sb[:, b, t * FREE : (t + 1) * FREE],
                            in_=pts[t], func=Copy,
                        )

            # ---- local stats -> (mean, E[x^2]) transposed to [2,128] ----
            pk = singles.tile([CO, 2], f32)
            sq = singles.tile([CO, 1], f32)
            nc.vector.bn_aggr(out=pk, in_=stats)
            nc.vector.tensor_mul(sq, pk[:, 0:1], pk[:, 0:1])
            nc.vector.tensor_add(pk[:, 1:2], pk[:, 1:2], sq)
            ptp = psum.tile([2, CO], f32, tag="pt")
            nc.tensor.transpose(ptp, pk, ident)
            pkT = singles.tile([2, CO], f32)
            nc.vector.tensor_copy(out=pkT, in_=ptp)

            # ---- AllGather [2,128] -> [16,128]; matmul-reduce over cores ----
            # cc_in staged from the gpsimd queue so the collective trigger
            # sits right behind it in the same queue
            cc_in = dram.tile([2, CO], f32)
            cc_out = dram.tile([2 * N_CORES, CO], f32)
            nc.gpsimd.dma_start(out=cc_in, in_=pkT)
            nc.gpsimd.collective_compute(
                "AllGather",
                mybir.AluOpType.bypass,
                replica_groups=[list(range(N_CORES))],
                ins=[cc_in[:].opt()],
                outs=[cc_out[:].opt()],
            )
            # land the gathered [16,128] in SBUF (16 descriptors) and reduce
            # over cores with ONE matmul: lhsT = gathered rows (row 2r =
            # mean_r, row 2r+1 = E2_r), moving = mask16 with 1/8 at (2r,0)
            # and (2r+1,1) -> PSUM [CO,2] = (gmean, E2avg) directly.
            g16 = singles.tile([16, CO], f32)
            nc.sync.dma_start(out=g16, in_=cc_out[:, :])
            pboth = psum.tile([CO, 2], f32, tag="pt", name="pboth")
            nc.tensor.matmul(pboth, lhsT=g16, rhs=mask16, start=True, stop=True)

            # a = rsqrt(var_M + EPS/4); shift = -mean_M * a.  Chain is
            # DVE(m2,gvar) -> ACT(sqrt) -> DVE(recip,shift): two engine
            # crossings on the critical path
            gmean = singles.tile([CO, 1], f32)
            m2 = singles.tile([CO, 1], f32)
            gvar = singles.tile([CO, 1], f32)
            sd = singles.tile([CO, 1], f32)
            a_sc = singles.tile([CO, 1], f32)
            shift = singles.tile([CO, 1], f32)
            nc.vector.tensor_copy(out=gmean, in_=pboth[:, 0:1])
            nc.vector.tensor_scalar(
                out=m2, in0=pboth[:, 0:1], scalar1=gmean[:, 0:1], scalar2=None,
                op0=ALU.mult,
            )
            nc.vector.tensor_scalar(
                out=gvar, in0=pboth[:, 1:2], scalar1=m2[:, 0:1], scalar2=None,
                op0=ALU.subtract,
            )
            eps_t = singles.tile([CO, 1], f32)
            nc.vector.memset(eps_t, EPS / 4.0)
            nc.scalar.activation(out=sd, in_=gvar, func=Sqrt, bias=eps_t[:, 0:1])
            nc.vector.reciprocal(a_sc, sd)
            nc.vector.tensor_scalar(
                out=shift, in0=a_sc, scalar1=gmean[:, 0:1],
                scalar2=-1.0, op0=ALU.mult, op1=ALU.mult,
            )

            # ---- normalize + store (DMA-bound; DVE/ACT produce) ----
            # normalize each store unit split DVE:ACT = 5:3 (matched so both
            # engines finish together); STORE mostly in 2048-col units (8 KiB
            # per-row descriptors are bandwidth-bound, smaller ones are bound
            # by ~200ns/descriptor queue processing).  First unit is small so
            # the store pipeline spins up early; last units are small so the
            # final store (and NEFF teardown behind it) starts early.
            UNITS = [512, 1536, 2048, 2048, 1024, 512, 512]
            for b in range(B_LOC):
                c0 = 0
                for ui, su in enumerate(UNITS):
                    dv = su * 5 // 8
                    sl_v = conv_sb[:, b, c0 : c0 + dv]
                    sl_a = conv_sb[:, b, c0 + dv : c0 + su]
                    nc.vector.tensor_scalar(
                        out=sl_v, in0=sl_v, scalar1=a_sc[:, 0:1],
                        scalar2=shift[:, 0:1], op0=ALU.mult, op1=ALU.add,
                    )
                    nc.scalar.activation(
                        out=sl_a, in_=sl_a, func=Ident,
                        bias=shift[:, 0:1], scale=a_sc[:, 0:1],
                    )
                    st = conv_sb[:, b, c0 : c0 + su]
                    # alternate trigger queues so one slow chunk can't
                    # head-of-line-block later ready stores
                    eng = (nc.gpsimd, nc.sync)[ui % 2]
                    eng.dma_start(out=out[b, :, c0 : c0 + su], in_=st)
                    c0 += su

    nc.compile()
    return nc


def _host_inputs(weight):
    from concourse import mybir

    bf16np = mybir.dt.np(mybir.dt.bfloat16)
    wT = np.sign(weight).transpose(1, 2, 0).astype(bf16np)  # [ci, k, co]
    ident = np.eye(128, dtype=np.float32)
    mask16 = np.zeros((16, 2), dtype=np.float32)
    mask16[0::2, 0] = 1.0 / N_CORES   # mean rows -> col 0
    mask16[1::2, 1] = 1.0 / N_CORES   # E2 rows   -> col 1
    return np.ascontiguousarray(wT), ident, mask16


def _run(inputs, trace=False):
    from concourse import bass_utils

    x = np.ascontiguousarray(np.asarray(inputs["x"], dtype=np.float32))
    weight = np.ascontiguousarray(np.asarray(inputs["weight"], dtype=np.float32))

    if "nc" not in _CACHE:
        _CACHE["nc"] = _build_nc()
    nc = _CACHE["nc"]

    wT, ident, mask16 = _host_inputs(weight)
    in_maps = [
        {
            "x": x[i * B_LOC : (i + 1) * B_LOC],
            "wT": wT,
            "ident": ident,
            "mask16": mask16,
        }
        for i in range(N_CORES)
    ]
    res = bass_utils.run_bass_kernel_spmd(
        nc, in_maps, core_ids=list(range(N_CORES)), trace=trace
    )
    out = np.concatenate(
        [res.results[i]["out"] for i in range(N_CORES)], axis=0
    ).astype(np.float32)
    return out, res


def kernel(**inputs) -> np.ndarray:
    out, _ = _run(inputs, trace=False)
    return out
